# revision 22
# baseline (speedup 1.0000x reference)
"""Chamfer distance kernel for Trainium2 (8 NeuronCores, SPMD).

Problem: pred [2, 8192, 3], gt [2, 8192, 3] (fp32) ->
  scalar = mean_b( mean_i min_j ||pred[b,j]-gt[b,i]|| + mean_j min_i ||...|| )

Strategy per core (gt rows sharded 8-way, per sharding hint):
  d2[i,j] = g2_i + p2_j - 2<g_i, p_j> is computed as a single K=5 matmul
  (padded to K=8) with extended vectors:
      S(g) = [g2, 1, -2gx, -2gy, -2gz]   (stationary / lhsT)
      T(p) = [1, p2, px, py, pz]         (streaming  / rhs)
  sqrt is monotonic so mins are taken on d2 and sqrt'd at the end.

  Each core owns 1024 gt rows per batch, streams all 8192 preds:
    - row-min over preds (dist1) is complete locally -> [B, 1024]
    - col-min over own gt rows (dist2 partial) -> [B, 8192], host min-reduces
      across the 8 cores.

  Engine split: PE produces d2 into PSUM ([128, 2048] groups); ScalarE casts
  PSUM fp32 -> SBUF fp16 (enables DVE 2x mode); DVE does fp16 tensor_tensor
  min folds for both directions; PE transposes the column accumulators so the
  partition-axis min becomes a free-axis tensor_reduce.
"""

import os
import sys

import numpy as np

for _p in ("/opt/trn_rl_repo", os.path.expanduser("~/.axon_site/_ro/trn_rl_repo")):
    if os.path.isdir(_p) and _p not in sys.path:
        sys.path.insert(0, _p)
        break

import concourse.bacc as bacc
import concourse.bass as bass
import concourse.tile as tile
from concourse import mybir
from concourse.masks import make_identity

FP32 = mybir.dt.float32
FP16 = mybir.dt.float16

N_CORES = 8
B = 2
N = 8192


def build_nc(
    batches: int,
    n_pred: int,
    n_gt_own: int,
    kp: int = 16,
    sup: int = 2048,
    fold_dt=FP16,
    mm_dt=FP16,
    mm_n: int = 512,
    packed: int = 4,
):
    """Build the per-core Bass program.

    DRAM I/O (per core):
      s_in  [kp, batches, n_gt_own]  - extended own gt rows (stationary, hi/lo)
      t_in  [kp, batches, n_pred]    - extended preds (streamed, replicated)
      rowmin_out [batches, n_gt_own//128, 128] fp32 - complete d2 row mins
      colmin_out [batches, n_pred//sup, sup//128, 128] fp32 - partial col mins
    """
    assert n_gt_own % 128 == 0 and n_pred % sup == 0 and sup % mm_n == 0
    gblk = n_gt_own // 128
    nsup = n_pred // sup
    tps = sup // 128
    qmm = sup // mm_n  # matmuls per psum group

    nc = bacc.Bacc()
    s_in = nc.dram_tensor("s_in", [kp, batches, n_gt_own], mm_dt, kind="ExternalInput")
    t_in = nc.dram_tensor("t_in", [kp, batches, n_pred], mm_dt, kind="ExternalInput")
    rowmin_out = nc.dram_tensor(
        "rowmin_out", [batches, gblk, 128], FP32, kind="ExternalOutput"
    )
    colmin_out = nc.dram_tensor(
        "colmin_out", [batches, nsup, tps, 128], FP32, kind="ExternalOutput"
    )

    with tile.TileContext(nc) as tc:
        with (
            tc.tile_pool(name="consts", bufs=1) as consts,
            tc.tile_pool(name="psum", bufs=2, space="PSUM") as psum,
            tc.tile_pool(name="casts", bufs=3) as casts,
            tc.tile_pool(name="rowaccs", bufs=2) as rowaccs,
            tc.tile_pool(name="colaccs", bufs=1) as colaccs,
            tc.tile_pool(name="coll", bufs=1) as coll,
        ):
            # Replicate S/T into 4 partition strips (0/32/64/96) so 4 matmuls
            # can run concurrently in distinct 32-row groups of the PE array.
            t_sb = consts.tile([128, batches, n_pred], mm_dt, tag="t_sb")
            s_sb = consts.tile([128, batches, n_gt_own], mm_dt, tag="s_sb")
            for q in range(4):
                nc.sync.dma_start(out=t_sb[32 * q : 32 * q + kp], in_=t_in[:])
                nc.sync.dma_start(out=s_sb[32 * q : 32 * q + kp], in_=s_in[:])

            ident = consts.tile([128, 128], fold_dt, tag="ident")
            make_identity(nc, ident)

            # HAM warmup: ~6us of dense back-to-back matmuls on garbage data
            # trips the PE clock gate to 8/8 (2.4 GHz) before the real work;
            # steady-state gaps stay below the ~3.4us MID window so it never
            # re-throttles. Results are never read.
            warm_in = consts.tile([kp, 512], mm_dt, tag="warm_in")
            nc.vector.memset(warm_in, 0.0)
            wps = psum.tile([128, sup], FP32, name="wps", tag="ps")
            for i in range(20):
                nc.tensor.matmul(
                    out=wps[:, (i % qmm) * mm_n : (i % qmm + 1) * mm_n],
                    lhsT=warm_in[:, 0:128],
                    rhs=warm_in[:, 0:mm_n],
                    start=True,
                    stop=True,
                )

            rowmin_coll = coll.tile([128, batches * gblk], FP32, tag="rowmin_coll")
            colmin_coll = coll.tile([128, batches, nsup, tps], FP32, tag="colmin_coll")

            # persistent column accumulators, one per (b, s)
            colacc = [
                [
                    colaccs.tile(
                        [128, sup],
                        fold_dt,
                        name=f"colacc_{b}_{s}",
                        tag=f"colacc_{b}_{s}",
                    )
                    for s in range(nsup)
                ]
                for b in range(batches)
            ]

            for b in range(batches):
                for g in range(gblk):
                    rowacc = rowaccs.tile([128, sup], fold_dt, tag="rowacc")
                    cast_tiles = []
                    for s in range(nsup):
                        ps = psum.tile([128, sup], FP32, tag="ps")
                        for q in range(qmm):
                            strip = (q % packed) * 32 if packed > 1 else 0
                            nc.tensor.matmul(
                                out=ps[:, q * mm_n : (q + 1) * mm_n],
                                lhsT=s_sb[
                                    strip : strip + kp, b, g * 128 : (g + 1) * 128
                                ],
                                rhs=t_sb[
                                    strip : strip + kp,
                                    b,
                                    s * sup + q * mm_n : s * sup + (q + 1) * mm_n,
                                ],
                                start=True,
                                stop=True,
                                tile_position=(strip, 0) if packed > 1 else None,
                            )
                        # cast PSUM fp32 -> SBUF fp16. For g==0 the cast output
                        # *is* the column accumulator (saves an init pass).
                        if g == 0:
                            cast_dst = colacc[b][s]
                        else:
                            cast_dst = casts.tile([128, sup], fold_dt, tag="cast")
                        nc.scalar.activation(
                            out=cast_dst,
                            in_=ps,
                            func=mybir.ActivationFunctionType.Copy,
                        )
                        # row fold (over pred blocks, for this gt block)
                        if s == 0:
                            cast_tiles.append(cast_dst)
                        elif s == 1:
                            nc.vector.tensor_tensor(
                                out=rowacc,
                                in0=cast_tiles[0],
                                in1=cast_dst,
                                op=mybir.AluOpType.min,
                            )
                        else:
                            nc.vector.tensor_tensor(
                                out=rowacc,
                                in0=rowacc,
                                in1=cast_dst,
                                op=mybir.AluOpType.min,
                            )
                        # column fold (over gt blocks, for this pred superblock)
                        if g > 0:
                            nc.vector.tensor_tensor(
                                out=colacc[b][s],
                                in0=colacc[b][s],
                                in1=cast_dst,
                                op=mybir.AluOpType.min,
                            )
                    if nsup == 1:
                        # degenerate: single superblock; reduce the cast directly
                        rowacc = cast_tiles[0]
                    nc.vector.tensor_reduce(
                        out=rowmin_coll[:, b * gblk + g : b * gblk + g + 1],
                        in_=rowacc,
                        axis=mybir.AxisListType.X,
                        op=mybir.AluOpType.min,
                    )
                    nc.sync.dma_start(
                        out=rowmin_out[b, g, :],
                        in_=rowmin_coll[:, b * gblk + g],
                    )

            # column-accumulator tails: PE-transpose the [128, sup] accumulator
            # in 128-wide chunks, then one grouped free-axis reduce gives the
            # min over the original partition axis for each pred.
            for b in range(batches):
                for s in range(nsup):
                    tp = psum.tile([128, sup], fold_dt, name="tp", tag="ps")
                    for c in range(tps):
                        nc.tensor.transpose(
                            out=tp[:, c * 128 : (c + 1) * 128],
                            in_=colacc[b][s][:, c * 128 : (c + 1) * 128],
                            identity=ident,
                        )
                    nc.vector.tensor_reduce(
                        out=colmin_coll[:, b, s, :],
                        in_=tp.rearrange("p (c q) -> p c q", q=128),
                        axis=mybir.AxisListType.X,
                        op=mybir.AluOpType.min,
                    )
                    nc.sync.dma_start(
                        out=colmin_out[b, s].rearrange("c p -> p c"),
                        in_=colmin_coll[:, b, s, :],
                    )
    nc.finalize()
    return nc


def t_in_slice(t_sb, b, start, width):
    return t_sb[:, b, start : start + width]


def _split_hl(x: np.ndarray):
    """fp32 -> (hi, lo) float16 pair with x ~= hi + lo."""
    hi = x.astype(np.float16)
    lo = (x - hi.astype(np.float32)).astype(np.float16)
    return hi, lo


def _pack_inputs(pred: np.ndarray, gt: np.ndarray, kp: int = 16):
    """Host-side shard prep: compensated hi/lo fp16 extended matrices.

    d2[i,j] = g2_i + p2_j - 2<g_i, p_j> is evaluated as a K=16 fp16 matmul
    with fp32 PSUM accumulation; each fp32 operand is split hi+lo and the
    three cross products (hi*hi, lo*hi, hi*lo) are packed into the K rows,
    so the only dropped term is lo*lo (~2^-22 relative).
    """
    pred = np.asarray(pred, dtype=np.float32)
    gt = np.asarray(gt, dtype=np.float32)
    bs, ng, _ = gt.shape
    _, npr, _ = pred.shape
    g2 = np.sum(gt * gt, axis=-1)  # [B, Ng]
    p2 = np.sum(pred * pred, axis=-1)  # [B, Np]
    m = -2.0 * gt  # [B, Ng, 3]
    g2h, g2l = _split_hl(g2)
    p2h, p2l = _split_hl(p2)
    mh, ml = _split_hl(m)
    ph, pl = _split_hl(pred)

    s_full = np.zeros((kp, bs, ng), dtype=np.float16)
    t_full = np.zeros((kp, bs, npr), dtype=np.float16)
    s_full[0], t_full[0] = g2h, 1.0
    s_full[1], t_full[1] = g2l, 1.0
    s_full[2], t_full[2] = 1.0, p2h
    s_full[3], t_full[3] = 1.0, p2l
    for d in range(3):
        s_full[4 + d], t_full[4 + d] = mh[..., d], ph[..., d]
        s_full[7 + d], t_full[7 + d] = ml[..., d], ph[..., d]
        s_full[10 + d], t_full[10 + d] = mh[..., d], pl[..., d]
    return s_full, t_full


_NC_CACHE = {}
PACKED = int(os.environ.get("CHAMFER_PACKED", "1"))


def _get_nc():
    key = (B, N, N // N_CORES, PACKED)
    if key not in _NC_CACHE:
        _NC_CACHE[key] = build_nc(B, N, N // N_CORES, packed=PACKED)
    return _NC_CACHE[key]


def _run_device(s_full, t_full, run_kwargs=None):
    from concourse.bass_utils import run_bass_kernel_spmd

    nc = _get_nc()
    own = N // N_CORES
    in_maps = [
        {
            "s_in": np.ascontiguousarray(s_full[:, :, c * own : (c + 1) * own]),
            "t_in": t_full,
        }
        for c in range(N_CORES)
    ]
    res = run_bass_kernel_spmd(
        nc, in_maps, core_ids=list(range(N_CORES)), **(run_kwargs or {})
    )
    return res


def _combine(results):
    own = N // N_CORES
    dist1_sq = np.empty((B, N), dtype=np.float32)
    colmins = []
    for c, out in enumerate(results):
        dist1_sq[:, c * own : (c + 1) * own] = out["rowmin_out"].reshape(B, own)
        colmins.append(out["colmin_out"].reshape(B, N))
    dist2_sq = np.min(np.stack(colmins, axis=0), axis=0)
    d1 = np.sqrt(np.maximum(dist1_sq.astype(np.float64), 0.0))
    d2 = np.sqrt(np.maximum(dist2_sq.astype(np.float64), 0.0))
    val = np.mean(np.mean(d1, axis=1) + np.mean(d2, axis=1))
    return np.float32(val)


def kernel(pred: np.ndarray, gt: np.ndarray) -> np.ndarray:
    s_full, t_full = _pack_inputs(pred, gt)
    res = _run_device(s_full, t_full)
    return _combine(res.results)


# revision 25
# speedup vs baseline: 1.2241x; 1.2241x over previous
"""Chamfer distance kernel for Trainium2 (8 NeuronCores, SPMD).

Problem: pred [2, 8192, 3], gt [2, 8192, 3] (fp32) ->
  scalar = mean_b( mean_i min_j ||pred[b,j]-gt[b,i]|| + mean_j min_i ||...|| )

Strategy per core (gt rows sharded 8-way, per sharding hint):
  d2[i,j] = g2_i + p2_j - 2<g_i, p_j> is computed as a single K=5 matmul
  (padded to K=8) with extended vectors:
      S(g) = [g2, 1, -2gx, -2gy, -2gz]   (stationary / lhsT)
      T(p) = [1, p2, px, py, pz]         (streaming  / rhs)
  sqrt is monotonic so mins are taken on d2 and sqrt'd at the end.

  Each core owns 1024 gt rows per batch, streams all 8192 preds:
    - row-min over preds (dist1) is complete locally -> [B, 1024]
    - col-min over own gt rows (dist2 partial) -> [B, 8192], host min-reduces
      across the 8 cores.

  Engine split: PE produces d2 into PSUM ([128, 2048] groups); ScalarE casts
  PSUM fp32 -> SBUF fp16 (enables DVE 2x mode); DVE does fp16 tensor_tensor
  min folds for both directions; PE transposes the column accumulators so the
  partition-axis min becomes a free-axis tensor_reduce.
"""

import os
import sys

import numpy as np

for _p in ("/opt/trn_rl_repo", os.path.expanduser("~/.axon_site/_ro/trn_rl_repo")):
    if os.path.isdir(_p) and _p not in sys.path:
        sys.path.insert(0, _p)
        break

import concourse.bacc as bacc
import concourse.bass as bass
import concourse.tile as tile
from concourse import mybir
from concourse.masks import make_identity

FP32 = mybir.dt.float32
FP16 = mybir.dt.float16

N_CORES = 8
B = 2
N = 8192


def build_nc(
    batches: int,
    n_pred: int,
    n_gt_own: int,
    kp: int = 16,
    sup: int = 2048,
    fold_dt=FP16,
    mm_dt=FP16,
    mm_n: int = 512,
    packed: int = 4,
):
    """Build the per-core Bass program.

    DRAM I/O (per core):
      s_in  [kp, batches, n_gt_own]  - extended own gt rows (stationary, hi/lo)
      t_in  [kp, batches, n_pred]    - extended preds (streamed, replicated)
      rowmin_out [batches, n_gt_own//128, 128] fp32 - complete d2 row mins
      colmin_out [batches, n_pred//sup, sup//128, 128] fp32 - partial col mins
    """
    assert n_gt_own % 128 == 0 and n_pred % sup == 0 and sup % mm_n == 0
    gblk = n_gt_own // 128
    nsup = n_pred // sup
    tps = sup // 128
    qmm = sup // mm_n  # matmuls per psum group

    nc = bacc.Bacc()
    s_in = nc.dram_tensor("s_in", [kp, batches, n_gt_own], mm_dt, kind="ExternalInput")
    t_in = nc.dram_tensor("t_in", [kp, batches, n_pred], mm_dt, kind="ExternalInput")
    # partition-major layouts so the output DMAs are contiguous per partition
    rowmin_out = nc.dram_tensor(
        "rowmin_out", [128, batches, gblk], FP32, kind="ExternalOutput"
    )
    colmin_out = nc.dram_tensor(
        "colmin_out", [128, batches, nsup, tps], FP32, kind="ExternalOutput"
    )

    with tile.TileContext(nc) as tc:
        with (
            tc.tile_pool(name="consts", bufs=1) as consts,
            tc.tile_pool(name="psum", bufs=2, space="PSUM") as psum,
            tc.tile_pool(name="casts", bufs=3) as casts,
            tc.tile_pool(name="rowaccs", bufs=2) as rowaccs,
            tc.tile_pool(name="colaccs", bufs=1) as colaccs,
            tc.tile_pool(name="coll", bufs=1) as coll,
        ):
            # Replicate S/T into 4 partition strips (0/32/64/96) so 4 matmuls
            # can run concurrently in distinct 32-row groups of the PE array.
            t_sb = consts.tile([128, batches, n_pred], mm_dt, tag="t_sb")
            s_sb = consts.tile([128, batches, n_gt_own], mm_dt, tag="s_sb")
            for q in range(4):
                nc.sync.dma_start(out=t_sb[32 * q : 32 * q + kp], in_=t_in[:])
                nc.sync.dma_start(out=s_sb[32 * q : 32 * q + kp], in_=s_in[:])

            ident = consts.tile([128, 128], fold_dt, tag="ident")
            make_identity(nc, ident)

            # HAM warmup: ~6us of dense back-to-back matmuls on garbage data
            # trips the PE clock gate to 8/8 (2.4 GHz) before the real work;
            # steady-state gaps stay below the ~3.4us MID window so it never
            # re-throttles. Results are never read.
            warm_in = consts.tile([kp, 512], mm_dt, tag="warm_in")
            nc.vector.memset(warm_in, 0.0)
            wps = psum.tile([128, sup], FP32, name="wps", tag="ps")
            for i in range(20):
                nc.tensor.matmul(
                    out=wps[:, (i % qmm) * mm_n : (i % qmm + 1) * mm_n],
                    lhsT=warm_in[:, 0:128],
                    rhs=warm_in[:, 0:mm_n],
                    start=True,
                    stop=True,
                )

            rowmin_coll = coll.tile([128, batches * gblk], FP32, tag="rowmin_coll")
            colmin_coll = coll.tile([128, batches, nsup, tps], FP32, tag="colmin_coll")

            # persistent column accumulators, one per (b, s)
            colacc = [
                [
                    colaccs.tile(
                        [128, sup],
                        fold_dt,
                        name=f"colacc_{b}_{s}",
                        tag=f"colacc_{b}_{s}",
                    )
                    for s in range(nsup)
                ]
                for b in range(batches)
            ]

            for b in range(batches):
                for g in range(gblk):
                    rowacc = rowaccs.tile([128, sup], fold_dt, tag="rowacc")
                    cast_tiles = []
                    for s in range(nsup):
                        ps = psum.tile([128, sup], FP32, tag="ps")
                        for q in range(qmm):
                            strip = (q % packed) * 32 if packed > 1 else 0
                            nc.tensor.matmul(
                                out=ps[:, q * mm_n : (q + 1) * mm_n],
                                lhsT=s_sb[
                                    strip : strip + kp, b, g * 128 : (g + 1) * 128
                                ],
                                rhs=t_sb[
                                    strip : strip + kp,
                                    b,
                                    s * sup + q * mm_n : s * sup + (q + 1) * mm_n,
                                ],
                                start=True,
                                stop=True,
                                tile_position=(strip, 0) if packed > 1 else None,
                            )
                        # cast PSUM fp32 -> SBUF fp16. For g==0 the cast output
                        # *is* the column accumulator (saves an init pass).
                        if g == 0:
                            cast_dst = colacc[b][s]
                        else:
                            cast_dst = casts.tile([128, sup], fold_dt, tag="cast")
                        nc.scalar.activation(
                            out=cast_dst,
                            in_=ps,
                            func=mybir.ActivationFunctionType.Copy,
                        )
                        # row fold (over pred blocks, for this gt block)
                        if s == 0:
                            cast_tiles.append(cast_dst)
                        elif s == 1:
                            nc.vector.tensor_tensor(
                                out=rowacc,
                                in0=cast_tiles[0],
                                in1=cast_dst,
                                op=mybir.AluOpType.min,
                            )
                        else:
                            nc.vector.tensor_tensor(
                                out=rowacc,
                                in0=rowacc,
                                in1=cast_dst,
                                op=mybir.AluOpType.min,
                            )
                        # column fold (over gt blocks, for this pred superblock)
                        if g > 0:
                            nc.vector.tensor_tensor(
                                out=colacc[b][s],
                                in0=colacc[b][s],
                                in1=cast_dst,
                                op=mybir.AluOpType.min,
                            )
                    if nsup == 1:
                        # degenerate: single superblock; reduce the cast directly
                        rowacc = cast_tiles[0]
                    nc.vector.tensor_reduce(
                        out=rowmin_coll[:, b * gblk + g : b * gblk + g + 1],
                        in_=rowacc,
                        axis=mybir.AxisListType.X,
                        op=mybir.AluOpType.min,
                    )

                # column-accumulator tail for this batch (overlaps the next
                # batch's main loop): PE-transpose the [128, sup] accumulators
                # in 128-wide chunks, then one grouped free-axis reduce gives
                # the min over the original partition axis for each pred.
                for s in range(nsup):
                    tp = psum.tile([128, sup], fold_dt, name="tp", tag="ps")
                    for c in range(tps):
                        nc.tensor.transpose(
                            out=tp[:, c * 128 : (c + 1) * 128],
                            in_=colacc[b][s][:, c * 128 : (c + 1) * 128],
                            identity=ident,
                        )
                    nc.vector.tensor_reduce(
                        out=colmin_coll[:, b, s, :],
                        in_=tp.rearrange("p (c q) -> p c q", q=128),
                        axis=mybir.AxisListType.X,
                        op=mybir.AluOpType.min,
                    )
                nc.sync.dma_start(
                    out=colmin_out[:, b], in_=colmin_coll[:, b]
                )
            nc.sync.dma_start(out=rowmin_out[:], in_=rowmin_coll)
    nc.finalize()
    return nc


def t_in_slice(t_sb, b, start, width):
    return t_sb[:, b, start : start + width]


def _split_hl(x: np.ndarray):
    """fp32 -> (hi, lo) float16 pair with x ~= hi + lo."""
    hi = x.astype(np.float16)
    lo = (x - hi.astype(np.float32)).astype(np.float16)
    return hi, lo


def _pack_inputs(pred: np.ndarray, gt: np.ndarray, kp: int = 16):
    """Host-side shard prep: compensated hi/lo fp16 extended matrices.

    d2[i,j] = g2_i + p2_j - 2<g_i, p_j> is evaluated as a K=16 fp16 matmul
    with fp32 PSUM accumulation; each fp32 operand is split hi+lo and the
    three cross products (hi*hi, lo*hi, hi*lo) are packed into the K rows,
    so the only dropped term is lo*lo (~2^-22 relative).
    """
    pred = np.asarray(pred, dtype=np.float32)
    gt = np.asarray(gt, dtype=np.float32)
    bs, ng, _ = gt.shape
    _, npr, _ = pred.shape
    g2 = np.sum(gt * gt, axis=-1)  # [B, Ng]
    p2 = np.sum(pred * pred, axis=-1)  # [B, Np]
    m = -2.0 * gt  # [B, Ng, 3]
    g2h, g2l = _split_hl(g2)
    p2h, p2l = _split_hl(p2)
    mh, ml = _split_hl(m)
    ph, pl = _split_hl(pred)

    s_full = np.zeros((kp, bs, ng), dtype=np.float16)
    t_full = np.zeros((kp, bs, npr), dtype=np.float16)
    s_full[0], t_full[0] = g2h, 1.0
    s_full[1], t_full[1] = g2l, 1.0
    s_full[2], t_full[2] = 1.0, p2h
    s_full[3], t_full[3] = 1.0, p2l
    for d in range(3):
        s_full[4 + d], t_full[4 + d] = mh[..., d], ph[..., d]
        s_full[7 + d], t_full[7 + d] = ml[..., d], ph[..., d]
        s_full[10 + d], t_full[10 + d] = mh[..., d], pl[..., d]
    return s_full, t_full


_NC_CACHE = {}
PACKED = int(os.environ.get("CHAMFER_PACKED", "1"))


def _get_nc():
    key = (B, N, N // N_CORES, PACKED)
    if key not in _NC_CACHE:
        _NC_CACHE[key] = build_nc(B, N, N // N_CORES, packed=PACKED)
    return _NC_CACHE[key]


def _run_device(s_full, t_full, run_kwargs=None):
    from concourse.bass_utils import run_bass_kernel_spmd

    nc = _get_nc()
    own = N // N_CORES
    in_maps = [
        {
            "s_in": np.ascontiguousarray(s_full[:, :, c * own : (c + 1) * own]),
            "t_in": t_full,
        }
        for c in range(N_CORES)
    ]
    res = run_bass_kernel_spmd(
        nc, in_maps, core_ids=list(range(N_CORES)), **(run_kwargs or {})
    )
    return res


def _combine(results):
    own = N // N_CORES
    dist1_sq = np.empty((B, N), dtype=np.float32)
    colmins = []
    for c, out in enumerate(results):
        # rowmin_out [128(p), B, gblk] -> [B, gblk*128]; local idx = g*128+p
        rm = out["rowmin_out"].transpose(1, 2, 0).reshape(B, own)
        dist1_sq[:, c * own : (c + 1) * own] = rm
        # colmin_out [128(p), B, nsup, tps] -> [B, N]; pred = s*sup + c*128 + p
        cm = out["colmin_out"].transpose(1, 2, 3, 0).reshape(B, N)
        colmins.append(cm)
    dist2_sq = np.min(np.stack(colmins, axis=0), axis=0)
    d1 = np.sqrt(np.maximum(dist1_sq.astype(np.float64), 0.0))
    d2 = np.sqrt(np.maximum(dist2_sq.astype(np.float64), 0.0))
    val = np.mean(np.mean(d1, axis=1) + np.mean(d2, axis=1))
    return np.float32(val)


def kernel(pred: np.ndarray, gt: np.ndarray) -> np.ndarray:
    s_full, t_full = _pack_inputs(pred, gt)
    res = _run_device(s_full, t_full)
    return _combine(res.results)
